# revision 12
# baseline (speedup 1.0000x reference)
"""Graphormer multi-head attention on 8 Trainium2 cores.

Sharding: 2 cores per batch element (B=4), each core owning 8 of 16 heads
(tensor-parallel within the batch).  v2 redesign over the baseline:

  - QK matmuls of a head PAIR are emitted adjacently: even head's K/Q live
    at partitions 0:64, odd head's at 64:128, so the two K=64 matmuls land
    on distinct PE row-groups and run concurrently (row tiling) -> ~2x QK.
  - All q/k/v biases are folded out of the device: bq/bk fold into the
    host-precomputed exp-bias matrix via rank-1 terms (c1[s], c2[t]),
    bv folds into a host-side constant (bv @ Wo.T) added at the end.
  - exp(scores) on Scalar; the bias multiply runs as scalar_tensor_tensor
    (bf16, all-SBUF) to hit the DVE 4x mode.
  - Host pre-packs every input into its exact SBUF layout so each DMA is a
    dense 2D copy, chunked so the first projection matmul starts ASAP.
  - Output projection is split into 4 per-fc waves, each emitted right
    after its head-pair's normalisation; partials ship as bf16 and are
    summed on the host.  PSUM choreography: scores tag (2x[128,1024]) +
    pv tag (2x[128,1024]) = 8 banks; mid-kernel y-waves reuse the pv tag,
    tail waves reuse the scores tag.
"""
import sys

sys.path.insert(0, '/opt/trn_rl_repo')

import ml_dtypes
import numpy as np

import concourse.bass as bass
import concourse.mybir as mybir
import concourse.tile as tile
from concourse import bacc
from concourse.bass_utils import run_bass_kernel_spmd

DT = mybir.dt
ALU = mybir.AluOpType

B, T, S, E, H = 4, 1024, 1024, 1024, 16
D = E // H          # 64
HL = 8              # heads per core
F = HL * D          # 512 local features
N_CORES = 8

MMDT = DT.bfloat16
NP_MMDT = ml_dtypes.bfloat16


def _build_program():
    nc = bacc.Bacc()

    xq = nc.dram_tensor("xq", [128, 8, T], MMDT, kind="ExternalInput")
    xk = nc.dram_tensor("xk", [128, 8, S], MMDT, kind="ExternalInput")
    xv = nc.dram_tensor("xv", [128, 8, S], MMDT, kind="ExternalInput")
    wq = nc.dram_tensor("wq", [128, 4, 8, 128], MMDT, kind="ExternalInput")
    wk = nc.dram_tensor("wk", [128, 4, 8, 128], MMDT, kind="ExternalInput")
    wv = nc.dram_tensor("wv", [128, 8, F], MMDT, kind="ExternalInput")
    wo = nc.dram_tensor("wo", [128, 4, E], MMDT, kind="ExternalInput")
    expb = nc.dram_tensor("expb", [128, 8, T], MMDT, kind="ExternalInput")
    bqd = nc.dram_tensor("bqd", [128, 4], DT.float32, kind="ExternalInput")
    bkd = nc.dram_tensor("bkd", [128, 4], DT.float32, kind="ExternalInput")
    youts = [nc.dram_tensor(f"y{w}", [E, T], MMDT, kind="ExternalOutput")
             for w in range(3)]

    with tile.TileContext(nc) as tc:
        with tc.tile_pool(name="persist", bufs=1) as pp, \
             tc.tile_pool(name="work", bufs=3) as wkp, \
             tc.tile_pool(name="big16", bufs=4) as big, \
             tc.tile_pool(name="psS", bufs=2, space="PSUM") as psS, \
             tc.tile_pool(name="psP", bufs=2, space="PSUM") as psP:

            # ---------------- DMA issues: ONE queue (sync), need-ordered ----
            # (multiple HWDGE queues share the same 16 DMA engines round-
            #  robin, so extra queues only dilute the critical path)
            wq_sb = pp.tile([128, 4, 8, 128], MMDT, tag="wq")
            xq_sb = big.tile([128, 8, T], MMDT, tag="b16")
            wk_sb = pp.tile([128, 4, 8, 128], MMDT, tag="wk")
            xk_sb = big.tile([128, 8, S], MMDT, tag="b16")
            xv_sb = big.tile([128, 8, S], MMDT, tag="b16")
            wv_sb = pp.tile([128, 8, F], MMDT, tag="wv")
            wo_sb = pp.tile([128, 4, E], MMDT, tag="wo")
            expb_sb = pp.tile([128, 8, T], MMDT, tag="expb")
            bq_sb = pp.tile([128, 4], DT.float32, tag="bq")
            bk_sb = pp.tile([128, 4], DT.float32, tag="bk")

            nc.sync.dma_start(bq_sb[:], bqd[:])
            nc.sync.dma_start(bk_sb[:], bkd[:])
            for half in range(2):  # first fc chunks split for an early start
                nc.sync.dma_start(wq_sb[:, 0, 4 * half:4 * half + 4],
                                  wq[:, 0, 4 * half:4 * half + 4])
            for half in range(2):
                nc.sync.dma_start(wk_sb[:, 0, 4 * half:4 * half + 4],
                                  wk[:, 0, 4 * half:4 * half + 4])
            for ec in range(8):
                nc.sync.dma_start(xq_sb[:, ec], xq[:, ec])
                nc.sync.dma_start(xk_sb[:, ec], xk[:, ec])
            for sc in range(2):
                nc.sync.dma_start(expb_sb[:, sc], expb[:, sc])
            for fc in range(1, 4):
                nc.sync.dma_start(wq_sb[:, fc], wq[:, fc])
                nc.sync.dma_start(wk_sb[:, fc], wk[:, fc])
            for ec in range(8):
                nc.sync.dma_start(xv_sb[:, ec], xv[:, ec])
            for sc in range(2, 8):
                nc.sync.dma_start(expb_sb[:, sc], expb[:, sc])
            for ec in range(8):
                nc.sync.dma_start(wv_sb[:, ec], wv[:, ec])
            for fc in range(4):
                nc.sync.dma_start(wo_sb[:, fc], wo[:, fc])

            # ---------------- persistent activation tiles ----
            qT_sb = pp.tile([128, 4, T], MMDT, tag="qT")
            kT_sb = pp.tile([128, 4, S], MMDT, tag="kT")
            v_sb = pp.tile([128, 8, HL, 65], MMDT, tag="v")
            nc.vector.memset(v_sb[:, :, :, 64:65], 1.0)
            oT_sb = pp.tile([128, 4, T], MMDT, tag="oT")

            state = {}

            # ---------------- q/k projections ----
            for fc in range(4):
                for x_sb, w_sb, b_sb, dst in ((xq_sb, wq_sb, bq_sb, qT_sb),
                                              (xk_sb, wk_sb, bk_sb, kT_sb)):
                    acc = psS.tile([128, T], DT.float32, tag="scores")
                    for th in range(2):
                        for ec in range(8):
                            nc.tensor.matmul(
                                acc[:, th * 512:(th + 1) * 512],
                                w_sb[:, fc, ec, :],
                                x_sb[:, ec, th * 512:(th + 1) * 512],
                                start=(ec == 0), stop=(ec == 7),
                            )
                    nc.vector.tensor_scalar_add(
                        dst[:, fc, :], acc[:], b_sb[:, fc:fc + 1])

            # ---------------- attention emission helpers ----
            def emit_vproj(sc):
                acc = psP.tile([128, F], DT.float32, tag="pv")
                for ec in range(8):
                    nc.tensor.matmul(
                        acc[:],
                        xv_sb[:, ec, sc * 128:(sc + 1) * 128],
                        wv_sb[:, ec, :],
                        start=(ec == 0), stop=(ec == 7),
                    )
                vv = v_sb[:, sc]
                nc.vector.tensor_copy(
                    vv[:, :, 0:64],
                    acc[:].rearrange("p (h d) -> p h d", d=64),
                )

            def emit_qk(p, sc):
                if sc == 0:
                    state[(p, "pT", 0)] = big.tile([128, 8, T], MMDT, tag="b16", name=f"pTe{p}")
                    state[(p, "pT", 1)] = big.tile([128, 8, T], MMDT, tag="b16", name=f"pTo{p}")
                sps = [psS.tile([128, T], DT.float32, tag="scores", name=f"sps{g}")
                       for g in range(2)]
                for th in range(2):
                    for g in range(2):  # even / odd head of the pair
                        po = 64 * g
                        nc.tensor.matmul(
                            sps[g][:, th * 512:(th + 1) * 512],
                            kT_sb[po:po + 64, p, sc * 128:(sc + 1) * 128],
                            qT_sb[po:po + 64, p, th * 512:(th + 1) * 512],
                            start=True, stop=True,
                        )
                for g in range(2):
                    if sc % 2 == 0:
                        state[("et", g)] = wkp.tile(
                            [128, 2, T], MMDT, tag="et", name=f"et{g}")
                    et = state[("et", g)]
                    nc.scalar.activation(et[:, sc % 2, :], sps[g][:],
                                         mybir.ActivationFunctionType.Exp)
                    if sc % 2 == 1:
                        nc.vector.tensor_mul(
                            state[(p, "pT", g)][:, sc - 1:sc + 1, :],
                            et[:], expb_sb[:, sc - 1:sc + 1, :],
                        )

            def emit_pv(p, i, tag="pv"):
                # i in 0..7; th = i//4, two sc' chunks per i per head
                if i == 0:
                    state[(p, "ops", 0)] = psP.tile([65, T], DT.float32, tag=tag, name=f"opse{p}")
                    state[(p, "ops", 1)] = psP.tile([65, T], DT.float32, tag=tag, name=f"opso{p}")
                for j in (2 * i, 2 * i + 1):
                    th, sc = divmod(j, 8)
                    for g in range(2):
                        nc.tensor.matmul(
                            state[(p, "ops", g)][:, th * 512:(th + 1) * 512],
                            v_sb[:, sc, 2 * p + g, :],
                            state[(p, "pT", g)][:, sc, th * 512:(th + 1) * 512],
                            start=(sc == 0), stop=(sc == 7),
                        )

            def emit_norm(p, th):
                if th is None:
                    lo, hi = 0, T
                else:
                    lo, hi = th * 512, (th + 1) * 512
                w = hi - lo
                for g in range(2):
                    ops = state[(p, "ops", g)]
                    po = 64 * g
                    lrow = wkp.tile([1, T], DT.float32, tag="lrow", bufs=2)
                    nc.vector.tensor_copy(lrow[:, 0:w], ops[64:65, lo:hi])
                    rl = wkp.tile([1, T], DT.float32, tag="rl", bufs=2)
                    nc.vector.reciprocal_approx_fast(
                        out=rl[:, 0:w], in_=lrow[:, 0:w])
                    rlb = wkp.tile([64, T], DT.float32, tag="rlb", bufs=2)
                    nc.gpsimd.partition_broadcast(rlb[:, 0:w], rl[:, 0:w])
                    nc.vector.tensor_mul(
                        oT_sb[po:po + 64, p, lo:hi], ops[0:64, lo:hi],
                        rlb[:, 0:w])

            ycopy_alt = [0]

            def emit_ywave_chunk(fcs, ec8, tag, w):
                pool = psS if tag == "scores" else psP
                yt = pool.tile([128, T], DT.float32, tag=tag, name=f"yt{w}_{ec8}")
                n = len(fcs)
                for th in range(2):
                    for i, fc in enumerate(fcs):
                        nc.tensor.matmul(
                            yt[:, th * 512:(th + 1) * 512],
                            wo_sb[:, fc, ec8 * 128:(ec8 + 1) * 128],
                            oT_sb[:, fc, th * 512:(th + 1) * 512],
                            start=(i == 0), stop=(i == n - 1),
                        )
                ys = wkp.tile([128, T], MMDT, tag="ys")
                if tag == "scores" and w == 2 and ycopy_alt[0] % 2 == 1:
                    nc.scalar.copy(ys[:], yt[:])  # tail: scalar is idle
                else:
                    nc.vector.tensor_copy(ys[:], yt[:])
                ycopy_alt[0] += 1
                nc.sync.dma_start(
                    youts[w][ec8 * 128:(ec8 + 1) * 128, :], ys[:])

            # ---------------- pair streams ----
            # pair 0: QK + v-proj fillers
            for sc in range(8):
                emit_qk(0, sc)
                emit_vproj(sc)

            # pairs 1..3: QK(p) + PV(p-1) + norms + y waves
            for p in range(1, 4):
                for sc in range(8):
                    emit_qk(p, sc)
                    emit_pv(p - 1, sc)
                    if sc == 4:
                        emit_norm(p - 1, 0)
                emit_norm(p - 1, 1)
                if p == 2:
                    for ec8 in range(8):
                        emit_ywave_chunk([0, 1], ec8, "pv", 0)
                if p == 3:
                    # scores tag: PV(3)'s pv-tag psum must not queue behind
                    # this wave's copies
                    for ec8 in range(8):
                        emit_ywave_chunk([2], ec8, "scores", 1)

            # tail: PV(3) + norms + wave fc3
            for i in range(8):
                emit_pv(3, i, tag="pv")
                if i == 4:
                    emit_norm(3, 0)
            emit_norm(3, 1)
            for ec8 in range(8):
                emit_ywave_chunk([3], ec8, "scores", 2)

    nc.compile()
    return nc


_NC_CACHE = []


def kernel(query, key_, value, edge_bias, attn_mask, key_padding_mask,
           Wq, bq, Wk, bk, Wv, bv, Wo, bo):
    if not _NC_CACHE:
        _NC_CACHE.append(_build_program())
    nc = _NC_CACHE[0]

    scale = np.float32(D ** -0.5)
    q32 = np.asarray(query, np.float32)
    k32 = np.asarray(key_, np.float32)
    v32 = np.asarray(value, np.float32)
    Wq32 = np.asarray(Wq, np.float32)
    Wk32 = np.asarray(Wk, np.float32)
    Wv32 = np.asarray(Wv, np.float32)
    Wo32 = np.asarray(Wo, np.float32)
    bq32 = np.asarray(bq, np.float32)
    bk32 = np.asarray(bk, np.float32)
    bv32 = np.asarray(bv, np.float32)
    bo32 = np.asarray(bo, np.float32)

    WqT = (Wq32.T * scale)          # [E, E] maps x -> scale*Wq x
    WkT = Wk32.T
    WvT = Wv32.T
    WoT = Wo32.T                    # [E(features), E(outputs)]

    kpm_add = np.where(np.asarray(key_padding_mask), np.float32(-1e30),
                       np.float32(0.0))          # [B, S]
    mask32 = np.asarray(attn_mask, np.float32)   # [T, S]

    def pack_x(xT):  # [E, T] -> [128, 8, T]
        return np.ascontiguousarray(
            xT.reshape(8, 128, T).transpose(1, 0, 2)).astype(NP_MMDT)

    def pack_w(wT, cols):  # [E, F-slice] -> [128, 4, 8, 128]
        w = wT[:, cols]                       # [E, 512]
        w = w.reshape(8, 128, 4, 128)         # ec, p, fc, m
        return np.ascontiguousarray(w.transpose(1, 2, 0, 3)).astype(NP_MMDT)

    in_maps = []
    for c in range(N_CORES):
        b, g = divmod(c, 2)
        cols = slice(g * F, (g + 1) * F)
        bias = (mask32.T + np.asarray(edge_bias[b], np.float32).T
                + kpm_add[b][:, None])
        wv_l = WvT[:, cols]                   # [E, 512]
        in_maps.append({
            "xq": pack_x(q32[b].T),
            "xk": pack_x(k32[b].T),
            "xv": pack_x(v32[b].T),
            "wq": pack_w(WqT, cols),
            "wk": pack_w(WkT, cols),
            "wv": np.ascontiguousarray(
                wv_l.reshape(8, 128, F).transpose(1, 0, 2)).astype(NP_MMDT),
            "wo": np.ascontiguousarray(
                WoT[cols, :].reshape(4, 128, E).transpose(1, 0, 2)
            ).astype(NP_MMDT),
            "expb": np.ascontiguousarray(
                np.exp(bias).reshape(8, 128, T).transpose(1, 0, 2)
            ).astype(NP_MMDT),
            "bqd": np.ascontiguousarray(
                (scale * bq32)[cols].reshape(4, 128).T),
            "bkd": np.ascontiguousarray(bk32[cols].reshape(4, 128).T),
        })

    res = run_bass_kernel_spmd(nc, in_maps, list(range(N_CORES)))

    ycst = (bv32 @ WoT + bo32).astype(np.float32)       # [E]
    out = np.empty((B, T, E), np.float32)
    for b in range(B):
        acc = np.zeros((E, T), np.float32)
        for c in (2 * b, 2 * b + 1):
            r = res.results[c]
            for w in range(3):
                acc += np.asarray(r[f"y{w}"], np.float32)
        out[b] = acc.T + ycst[None, :]
    return out


# revision 13
# speedup vs baseline: 1.0431x; 1.0431x over previous
"""Graphormer multi-head attention on 8 Trainium2 cores.

Sharding: 2 cores per batch element (B=4), each core owning 8 of 16 heads
(tensor-parallel within the batch).  Design notes (v6):

  - The exp chain is the critical path: 64 [128,1024] Exp activations on
    the Scalar engine (~1.1us each).  Everything else is arranged to hide
    under it: QK pairs feed scores through a 2-buffer PSUM rotation,
    elementwise work is balanced across DVE and Scalar, PV/projection/
    output matmuls fill the PE between QK bursts.
  - PSUM choreography (8 banks): psS "scores" = 2x[128,1024] fp32 used
    ONLY by QK scores (+ tail y-waves), so pair-0 attention starts the
    moment q/k fc0 projections finish and overlaps the DMA-paced rest of
    the projection phase.  psP "pv" = 2x[128,1024] carries proj
    accumulators -> v-proj -> PV accumulators -> mid y-waves in FIFO
    order that matches natural data readiness.
  - bq/bk ride the projection PSUM->SBUF transfer (per-partition scalar
    add); bv folds into a host-side constant (bv @ Wo.T) added at the
    end; exp(attn_mask + edge_bias) multiplies as bf16 (DVE 2x mode),
    merged over 2 score chunks per instruction.
  - Host pre-packs every input into its exact SBUF layout; one
    need-ordered DMA queue (extra queues just dilute the shared 16 DMA
    engines).  Output projection ships as 3 bf16 partials summed on host.
"""
import sys

sys.path.insert(0, '/opt/trn_rl_repo')

import ml_dtypes
import numpy as np

import concourse.bass as bass
import concourse.mybir as mybir
import concourse.tile as tile
from concourse import bacc
from concourse.bass_utils import run_bass_kernel_spmd

DT = mybir.dt
ALU = mybir.AluOpType

B, T, S, E, H = 4, 1024, 1024, 1024, 16
D = E // H          # 64
HL = 8              # heads per core
F = HL * D          # 512 local features
N_CORES = 8

MMDT = DT.bfloat16
NP_MMDT = ml_dtypes.bfloat16


def _build_program():
    nc = bacc.Bacc()

    xq = nc.dram_tensor("xq", [128, 8, T], MMDT, kind="ExternalInput")
    xk = nc.dram_tensor("xk", [128, 8, S], MMDT, kind="ExternalInput")
    xv = nc.dram_tensor("xv", [128, 8, S], MMDT, kind="ExternalInput")
    wq = nc.dram_tensor("wq", [128, 4, 8, 128], MMDT, kind="ExternalInput")
    wk = nc.dram_tensor("wk", [128, 4, 8, 128], MMDT, kind="ExternalInput")
    wv = nc.dram_tensor("wv", [128, 8, F], MMDT, kind="ExternalInput")
    wo = nc.dram_tensor("wo", [128, 4, E], MMDT, kind="ExternalInput")
    expb = nc.dram_tensor("expb", [128, 8, T], MMDT, kind="ExternalInput")
    bqd = nc.dram_tensor("bqd", [128, 4], DT.float32, kind="ExternalInput")
    bkd = nc.dram_tensor("bkd", [128, 4], DT.float32, kind="ExternalInput")
    youts = [nc.dram_tensor(f"y{w}", [E, T], MMDT, kind="ExternalOutput")
             for w in range(3)]

    with tile.TileContext(nc) as tc:
        with tc.tile_pool(name="persist", bufs=1) as pp, \
             tc.tile_pool(name="work", bufs=3) as wkp, \
             tc.tile_pool(name="big16", bufs=5) as big, \
             tc.tile_pool(name="psS", bufs=2, space="PSUM") as psS, \
             tc.tile_pool(name="psP", bufs=2, space="PSUM") as psP:

            # ---------------- DMA issues: ONE queue (sync), need-ordered ----
            wq_sb = pp.tile([128, 4, 8, 128], MMDT, tag="wq")
            xq_sb = big.tile([128, 8, T], MMDT, tag="b16")
            wk_sb = pp.tile([128, 4, 8, 128], MMDT, tag="wk")
            xk_sb = big.tile([128, 8, S], MMDT, tag="b16")
            xv_sb = big.tile([128, 8, S], MMDT, tag="b16")
            wv_sb = pp.tile([128, 8, F], MMDT, tag="wv")
            wo_sb = pp.tile([128, 4, E], MMDT, tag="wo")
            expb_sb = pp.tile([128, 8, T], MMDT, tag="expb")
            bq_sb = pp.tile([128, 4], DT.float32, tag="bq")
            bk_sb = pp.tile([128, 4], DT.float32, tag="bk")

            nc.sync.dma_start(bq_sb[:], bqd[:])
            nc.sync.dma_start(bk_sb[:], bkd[:])
            for half in range(2):
                nc.sync.dma_start(wq_sb[:, 0, 4 * half:4 * half + 4],
                                  wq[:, 0, 4 * half:4 * half + 4])
            for half in range(2):
                nc.sync.dma_start(wk_sb[:, 0, 4 * half:4 * half + 4],
                                  wk[:, 0, 4 * half:4 * half + 4])
            for ec in range(8):
                nc.sync.dma_start(xq_sb[:, ec], xq[:, ec])
                nc.sync.dma_start(xk_sb[:, ec], xk[:, ec])
            for sc in range(2):
                nc.sync.dma_start(expb_sb[:, sc], expb[:, sc])
            for fc in range(1, 4):
                nc.sync.dma_start(wq_sb[:, fc], wq[:, fc])
                nc.sync.dma_start(wk_sb[:, fc], wk[:, fc])
            for ec in range(8):
                nc.sync.dma_start(xv_sb[:, ec], xv[:, ec])
            for sc in range(2, 8):
                nc.sync.dma_start(expb_sb[:, sc], expb[:, sc])
            for ec in range(8):
                nc.sync.dma_start(wv_sb[:, ec], wv[:, ec])
            for fc in range(4):
                nc.sync.dma_start(wo_sb[:, fc], wo[:, fc])

            # ---------------- persistent activation tiles ----
            qT_sb = pp.tile([128, 4, T], MMDT, tag="qT")
            kT_sb = pp.tile([128, 4, S], MMDT, tag="kT")
            v_sb = pp.tile([128, 8, HL, 65], MMDT, tag="v")
            nc.vector.memset(v_sb[:, :, :, 64:65], 1.0)
            oT_sb = pp.tile([128, 4, T], MMDT, tag="oT")

            state = {}

            # ---------------- emission helpers ----
            def emit_proj(which, fc):
                x_sb, w_sb, b_sb, dst = {
                    "q": (xq_sb, wq_sb, bq_sb, qT_sb),
                    "k": (xk_sb, wk_sb, bk_sb, kT_sb),
                }[which]
                acc = psP.tile([128, T], DT.float32, tag="pv",
                               name=f"prj{which}{fc}")
                for th in range(2):
                    for ec in range(8):
                        nc.tensor.matmul(
                            acc[:, th * 512:(th + 1) * 512],
                            w_sb[:, fc, ec, :],
                            x_sb[:, ec, th * 512:(th + 1) * 512],
                            start=(ec == 0), stop=(ec == 7),
                        )
                nc.vector.tensor_scalar_add(
                    dst[:, fc, :], acc[:], b_sb[:, fc:fc + 1])

            def emit_vproj(sc):
                acc = psP.tile([128, F], DT.float32, tag="pv", name=f"vp{sc}")
                for ec in range(8):
                    nc.tensor.matmul(
                        acc[:],
                        xv_sb[:, ec, sc * 128:(sc + 1) * 128],
                        wv_sb[:, ec, :],
                        start=(ec == 0), stop=(ec == 7),
                    )
                nc.vector.tensor_copy(
                    v_sb[:, sc, :, 0:64],
                    acc[:].rearrange("p (h d) -> p h d", d=64),
                )

            def emit_qk(p, sc):
                if sc == 0:
                    state[(p, "pT", 0)] = big.tile([128, 8, T], MMDT,
                                                   tag="b16", name=f"pTe{p}")
                    state[(p, "pT", 1)] = big.tile([128, 8, T], MMDT,
                                                   tag="b16", name=f"pTo{p}")
                sps = [psS.tile([128, T], DT.float32, tag="scores",
                                name=f"sps{g}") for g in range(2)]
                for th in range(2):
                    for g in range(2):  # even / odd head of the pair
                        po = 64 * g
                        nc.tensor.matmul(
                            sps[g][:, th * 512:(th + 1) * 512],
                            kT_sb[po:po + 64, p, sc * 128:(sc + 1) * 128],
                            qT_sb[po:po + 64, p, th * 512:(th + 1) * 512],
                            start=True, stop=True,
                        )
                for g in range(2):
                    if sc % 2 == 0:
                        state[("et", g)] = wkp.tile(
                            [128, 2, T], MMDT, tag="et", bufs=4,
                            name=f"et{g}")
                    et = state[("et", g)]
                    nc.scalar.activation(et[:, sc % 2, :], sps[g][:],
                                         mybir.ActivationFunctionType.Exp)
                    if sc % 2 == 1:
                        nc.vector.tensor_mul(
                            state[(p, "pT", g)][:, sc - 1:sc + 1, :],
                            et[:], expb_sb[:, sc - 1:sc + 1, :],
                        )

            def emit_pv(p, i):
                if i == 0:
                    state[(p, "ops", 0)] = psP.tile([65, T], DT.float32,
                                                    tag="pv", name=f"opse{p}")
                    state[(p, "ops", 1)] = psP.tile([65, T], DT.float32,
                                                    tag="pv", name=f"opso{p}")
                for j in (2 * i, 2 * i + 1):
                    th, sc = divmod(j, 8)
                    for g in range(2):
                        nc.tensor.matmul(
                            state[(p, "ops", g)][:, th * 512:(th + 1) * 512],
                            v_sb[:, sc, 2 * p + g, :],
                            state[(p, "pT", g)][:, sc, th * 512:(th + 1) * 512],
                            start=(sc == 0), stop=(sc == 7),
                        )

            def emit_norm(p, th):
                lo, hi = th * 512, (th + 1) * 512
                for g in range(2):
                    ops = state[(p, "ops", g)]
                    po = 64 * g
                    lrow = wkp.tile([1, 512], DT.float32, tag="lrow", bufs=2)
                    nc.vector.tensor_copy(lrow[:], ops[64:65, lo:hi])
                    rl = wkp.tile([1, 512], DT.float32, tag="rl", bufs=2)
                    nc.vector.reciprocal_approx_fast(out=rl[:], in_=lrow[:])
                    rlb = wkp.tile([64, 512], DT.float32, tag="rlb", bufs=2)
                    nc.gpsimd.partition_broadcast(rlb[:], rl[:])
                    nc.vector.tensor_mul(
                        oT_sb[po:po + 64, p, lo:hi], ops[0:64, lo:hi], rlb[:])

            ycopy_alt = [0]

            def emit_ywave_chunk(fcs, ec8, tag, w):
                pool = psS if tag == "scores" else psP
                yt = pool.tile([128, T], DT.float32, tag=tag,
                               name=f"yt{w}_{ec8}")
                n = len(fcs)
                for th in range(2):
                    for i, fc in enumerate(fcs):
                        nc.tensor.matmul(
                            yt[:, th * 512:(th + 1) * 512],
                            wo_sb[:, fc, ec8 * 128:(ec8 + 1) * 128],
                            oT_sb[:, fc, th * 512:(th + 1) * 512],
                            start=(i == 0), stop=(i == n - 1),
                        )
                ys = wkp.tile([128, T], MMDT, tag="ys")
                if ycopy_alt[0] % 2 == 1:
                    nc.scalar.copy(ys[:], yt[:])
                else:
                    nc.vector.tensor_copy(ys[:], yt[:])
                ycopy_alt[0] += 1
                nc.sync.dma_start(
                    youts[w][ec8 * 128:(ec8 + 1) * 128, :], ys[:])

            # ---------------- pair streams ----
            # pair 0 overlaps the remaining projections (fc1-3 + v-proj)
            emit_proj("q", 0)
            emit_proj("k", 0)
            fillers = [("p", "q", 1), ("p", "k", 1), ("p", "q", 2),
                       ("p", "k", 2), ("p", "q", 3), ("p", "k", 3)] + \
                      [("v", s) for s in range(8)]
            fi = 0
            for sc in range(8):
                emit_qk(0, sc)
                take = 2 if sc < 6 else 1
                for _ in range(take):
                    if fi < len(fillers):
                        fl = fillers[fi]; fi += 1
                        if fl[0] == "p":
                            emit_proj(fl[1], fl[2])
                        else:
                            emit_vproj(fl[1])
            while fi < len(fillers):
                fl = fillers[fi]; fi += 1
                if fl[0] == "p":
                    emit_proj(fl[1], fl[2])
                else:
                    emit_vproj(fl[1])

            # pairs 1..3: QK(p) + PV(p-1) + norms + y waves
            for p in range(1, 4):
                for sc in range(8):
                    emit_qk(p, sc)
                    emit_pv(p - 1, sc)
                    if sc == 4:
                        emit_norm(p - 1, 0)
                emit_norm(p - 1, 1)
                if p == 2:
                    for ec8 in range(8):
                        emit_ywave_chunk([0, 1], ec8, "pv", 0)
                if p == 3:
                    # scores tag: PV(3)'s pv-tag psum must not queue behind
                    # this wave's copies
                    for ec8 in range(8):
                        emit_ywave_chunk([2], ec8, "scores", 1)

            # tail: PV(3) + norms + wave fc3
            for i in range(8):
                emit_pv(3, i)
                if i == 4:
                    emit_norm(3, 0)
            emit_norm(3, 1)
            for ec8 in range(8):
                emit_ywave_chunk([3], ec8, "scores", 2)

    nc.compile()
    return nc


_NC_CACHE = []


def kernel(query, key_, value, edge_bias, attn_mask, key_padding_mask,
           Wq, bq, Wk, bk, Wv, bv, Wo, bo):
    if not _NC_CACHE:
        _NC_CACHE.append(_build_program())
    nc = _NC_CACHE[0]

    scale = np.float32(D ** -0.5)
    q32 = np.asarray(query, np.float32)
    k32 = np.asarray(key_, np.float32)
    v32 = np.asarray(value, np.float32)
    Wq32 = np.asarray(Wq, np.float32)
    Wk32 = np.asarray(Wk, np.float32)
    Wv32 = np.asarray(Wv, np.float32)
    Wo32 = np.asarray(Wo, np.float32)
    bq32 = np.asarray(bq, np.float32)
    bk32 = np.asarray(bk, np.float32)
    bv32 = np.asarray(bv, np.float32)
    bo32 = np.asarray(bo, np.float32)

    WqT = (Wq32.T * scale)
    WkT = Wk32.T
    WvT = Wv32.T
    WoT = Wo32.T

    kpm_add = np.where(np.asarray(key_padding_mask), np.float32(-1e30),
                       np.float32(0.0))          # [B, S]
    mask32 = np.asarray(attn_mask, np.float32)   # [T, S]

    def pack_x(xT):  # [E, T] -> [128, 8, T]
        return np.ascontiguousarray(
            xT.reshape(8, 128, T).transpose(1, 0, 2)).astype(NP_MMDT)

    def pack_w(wT, cols):  # [E, F-slice] -> [128, 4, 8, 128]
        w = wT[:, cols]
        w = w.reshape(8, 128, 4, 128)
        return np.ascontiguousarray(w.transpose(1, 2, 0, 3)).astype(NP_MMDT)

    in_maps = []
    for c in range(N_CORES):
        b, g = divmod(c, 2)
        cols = slice(g * F, (g + 1) * F)
        bias = (mask32.T + np.asarray(edge_bias[b], np.float32).T
                + kpm_add[b][:, None])
        wv_l = WvT[:, cols]
        in_maps.append({
            "xq": pack_x(q32[b].T),
            "xk": pack_x(k32[b].T),
            "xv": pack_x(v32[b].T),
            "wq": pack_w(WqT, cols),
            "wk": pack_w(WkT, cols),
            "wv": np.ascontiguousarray(
                wv_l.reshape(8, 128, F).transpose(1, 0, 2)).astype(NP_MMDT),
            "wo": np.ascontiguousarray(
                WoT[cols, :].reshape(4, 128, E).transpose(1, 0, 2)
            ).astype(NP_MMDT),
            "expb": np.ascontiguousarray(
                np.exp(bias).reshape(8, 128, T).transpose(1, 0, 2)
            ).astype(NP_MMDT),
            "bqd": np.ascontiguousarray(
                (scale * bq32)[cols].reshape(4, 128).T),
            "bkd": np.ascontiguousarray(bk32[cols].reshape(4, 128).T),
        })

    res = run_bass_kernel_spmd(nc, in_maps, list(range(N_CORES)))

    ycst = (bv32 @ WoT + bo32).astype(np.float32)
    out = np.empty((B, T, E), np.float32)
    for b in range(B):
        acc = np.zeros((E, T), np.float32)
        for c in (2 * b, 2 * b + 1):
            r = res.results[c]
            for w in range(3):
                acc += np.asarray(r[f"y{w}"], np.float32)
        out[b] = acc.T + ycst[None, :]
    return out


# revision 14
# speedup vs baseline: 1.1142x; 1.0681x over previous
"""Graphormer multi-head attention on 8 Trainium2 cores.

Sharding: 2 cores per batch element (B=4), each core owning 8 of 16 heads
(tensor-parallel within the batch).  Design notes (v6):

  - The exp chain is the critical path: 64 [128,1024] Exp activations on
    the Scalar engine (~1.1us each).  Everything else is arranged to hide
    under it: QK pairs feed scores through a 2-buffer PSUM rotation,
    elementwise work is balanced across DVE and Scalar, PV/projection/
    output matmuls fill the PE between QK bursts.
  - PSUM choreography (8 banks): psS "scores" = 2x[128,1024] fp32 used
    ONLY by QK scores (+ tail y-waves), so pair-0 attention starts the
    moment q/k fc0 projections finish and overlaps the DMA-paced rest of
    the projection phase.  psP "pv" = 2x[128,1024] carries proj
    accumulators -> v-proj -> PV accumulators -> mid y-waves in FIFO
    order that matches natural data readiness.
  - bq/bk ride the projection PSUM->SBUF transfer (per-partition scalar
    add); bv folds into a host-side constant (bv @ Wo.T) added at the
    end; exp(attn_mask + edge_bias) multiplies as bf16 (DVE 2x mode),
    merged over 2 score chunks per instruction.
  - Host pre-packs every input into its exact SBUF layout; one
    need-ordered DMA queue (extra queues just dilute the shared 16 DMA
    engines).  Output projection ships as 3 bf16 partials summed on host.
"""
import sys

sys.path.insert(0, '/opt/trn_rl_repo')

import ml_dtypes
import numpy as np

import concourse.bass as bass
import concourse.mybir as mybir
import concourse.tile as tile
from concourse import bacc
from concourse.bass_utils import run_bass_kernel_spmd

DT = mybir.dt
ALU = mybir.AluOpType

B, T, S, E, H = 4, 1024, 1024, 1024, 16
D = E // H          # 64
HL = 8              # heads per core
F = HL * D          # 512 local features
N_CORES = 8

MMDT = DT.bfloat16
NP_MMDT = ml_dtypes.bfloat16


def _build_program():
    nc = bacc.Bacc()

    xq = nc.dram_tensor("xq", [128, 8, T], MMDT, kind="ExternalInput")
    xk = nc.dram_tensor("xk", [128, 8, S], MMDT, kind="ExternalInput")
    xv = nc.dram_tensor("xv", [128, 8, S], MMDT, kind="ExternalInput")
    wq = nc.dram_tensor("wq", [128, 4, 8, 128], MMDT, kind="ExternalInput")
    wk = nc.dram_tensor("wk", [128, 4, 8, 128], MMDT, kind="ExternalInput")
    wv = nc.dram_tensor("wv", [128, 8, F], MMDT, kind="ExternalInput")
    wo = nc.dram_tensor("wo", [128, 4, E], MMDT, kind="ExternalInput")
    expb = nc.dram_tensor("expb", [128, 8, T], MMDT, kind="ExternalInput")
    bqd = nc.dram_tensor("bqd", [128, 4], DT.float32, kind="ExternalInput")
    bkd = nc.dram_tensor("bkd", [128, 4], DT.float32, kind="ExternalInput")
    youts = [nc.dram_tensor(f"y{w}", [E, T], MMDT, kind="ExternalOutput")
             for w in range(3)]

    with tile.TileContext(nc) as tc:
        with tc.tile_pool(name="persist", bufs=1) as pp, \
             tc.tile_pool(name="work", bufs=3) as wkp, \
             tc.tile_pool(name="big16", bufs=5) as big, \
             tc.tile_pool(name="psS", bufs=2, space="PSUM") as psS, \
             tc.tile_pool(name="psP", bufs=2, space="PSUM") as psP:

            # ---------------- DMA issues: ONE queue (sync), need-ordered ----
            wq_sb = pp.tile([128, 4, 8, 128], MMDT, tag="wq")
            xq_sb = big.tile([128, 8, T], MMDT, tag="b16")
            wk_sb = pp.tile([128, 4, 8, 128], MMDT, tag="wk")
            xk_sb = big.tile([128, 8, S], MMDT, tag="b16")
            xv_sb = big.tile([128, 8, S], MMDT, tag="b16")
            wv_sb = pp.tile([128, 8, F], MMDT, tag="wv")
            wo_sb = pp.tile([128, 4, E], MMDT, tag="wo")
            expb_sb = pp.tile([128, 8, T], MMDT, tag="expb")
            bq_sb = pp.tile([128, 4], DT.float32, tag="bq")
            bk_sb = pp.tile([128, 4], DT.float32, tag="bk")

            nc.sync.dma_start(bq_sb[:], bqd[:])
            nc.sync.dma_start(bk_sb[:], bkd[:])
            for half in range(2):
                nc.sync.dma_start(wq_sb[:, 0, 4 * half:4 * half + 4],
                                  wq[:, 0, 4 * half:4 * half + 4])
            for half in range(2):
                nc.sync.dma_start(wk_sb[:, 0, 4 * half:4 * half + 4],
                                  wk[:, 0, 4 * half:4 * half + 4])
            for ec in range(8):
                nc.sync.dma_start(xq_sb[:, ec], xq[:, ec])
                nc.sync.dma_start(xk_sb[:, ec], xk[:, ec])
            for sc in range(2):
                nc.sync.dma_start(expb_sb[:, sc], expb[:, sc])
            for fc in range(1, 4):
                nc.sync.dma_start(wq_sb[:, fc], wq[:, fc])
                nc.sync.dma_start(wk_sb[:, fc], wk[:, fc])
            for ec in range(8):
                nc.sync.dma_start(xv_sb[:, ec], xv[:, ec])
            for sc in range(2, 8):
                nc.sync.dma_start(expb_sb[:, sc], expb[:, sc])
            for ec in range(8):
                nc.sync.dma_start(wv_sb[:, ec], wv[:, ec])
            for fc in range(4):
                nc.sync.dma_start(wo_sb[:, fc], wo[:, fc])

            # ---------------- persistent activation tiles ----
            qT_sb = pp.tile([128, 4, T], MMDT, tag="qT")
            kT_sb = pp.tile([128, 4, S], MMDT, tag="kT")
            v_sb = pp.tile([128, 8, HL, 65], MMDT, tag="v")
            nc.vector.memset(v_sb[:, :, :, 64:65], 1.0)
            oT_sb = pp.tile([128, 4, T], MMDT, tag="oT")

            state = {}

            # ---------------- emission helpers ----
            def emit_proj(which, fc):
                x_sb, w_sb, b_sb, dst = {
                    "q": (xq_sb, wq_sb, bq_sb, qT_sb),
                    "k": (xk_sb, wk_sb, bk_sb, kT_sb),
                }[which]
                acc = psP.tile([128, T], DT.float32, tag="pv",
                               name=f"prj{which}{fc}")
                for th in range(2):
                    for ec in range(8):
                        nc.tensor.matmul(
                            acc[:, th * 512:(th + 1) * 512],
                            w_sb[:, fc, ec, :],
                            x_sb[:, ec, th * 512:(th + 1) * 512],
                            start=(ec == 0), stop=(ec == 7),
                        )
                nc.vector.tensor_scalar_add(
                    dst[:, fc, :], acc[:], b_sb[:, fc:fc + 1])

            def emit_vproj(sc):
                acc = psP.tile([128, F], DT.float32, tag="pv", name=f"vp{sc}")
                for ec in range(8):
                    nc.tensor.matmul(
                        acc[:],
                        xv_sb[:, ec, sc * 128:(sc + 1) * 128],
                        wv_sb[:, ec, :],
                        start=(ec == 0), stop=(ec == 7),
                    )
                nc.vector.tensor_copy(
                    v_sb[:, sc, :, 0:64],
                    acc[:].rearrange("p (h d) -> p h d", d=64),
                )

            def emit_qk(p, sc):
                if sc == 0:
                    state[(p, "pT", 0)] = big.tile([128, 8, T], MMDT,
                                                   tag="b16", name=f"pTe{p}")
                    state[(p, "pT", 1)] = big.tile([128, 8, T], MMDT,
                                                   tag="b16", name=f"pTo{p}")
                sps = [psS.tile([128, T], DT.float32, tag="scores",
                                name=f"sps{g}") for g in range(2)]
                for th in range(2):
                    for g in range(2):  # even / odd head of the pair
                        po = 64 * g
                        nc.tensor.matmul(
                            sps[g][:, th * 512:(th + 1) * 512],
                            kT_sb[po:po + 64, p, sc * 128:(sc + 1) * 128],
                            qT_sb[po:po + 64, p, th * 512:(th + 1) * 512],
                            start=True, stop=True,
                        )
                for g in range(2):
                    if sc % 2 == 0:
                        state[("et", g)] = wkp.tile(
                            [128, 2, T], MMDT, tag="et", bufs=4,
                            name=f"et{g}")
                    et = state[("et", g)]
                    nc.scalar.activation(et[:, sc % 2, :], sps[g][:],
                                         mybir.ActivationFunctionType.Exp)
                    if sc % 2 == 1:
                        nc.vector.tensor_mul(
                            state[(p, "pT", g)][:, sc - 1:sc + 1, :],
                            et[:], expb_sb[:, sc - 1:sc + 1, :],
                        )

            def emit_pv(p, i):
                if i == 0:
                    state[(p, "ops", 0)] = psP.tile([65, T], DT.float32,
                                                    tag="pv", name=f"opse{p}")
                    state[(p, "ops", 1)] = psP.tile([65, T], DT.float32,
                                                    tag="pv", name=f"opso{p}")
                for j in (2 * i, 2 * i + 1):
                    th, sc = divmod(j, 8)
                    for g in range(2):
                        nc.tensor.matmul(
                            state[(p, "ops", g)][:, th * 512:(th + 1) * 512],
                            v_sb[:, sc, 2 * p + g, :],
                            state[(p, "pT", g)][:, sc, th * 512:(th + 1) * 512],
                            start=(sc == 0), stop=(sc == 7),
                        )

            def emit_norm(p, th):
                lo, hi = th * 512, (th + 1) * 512
                for g in range(2):
                    ops = state[(p, "ops", g)]
                    po = 64 * g
                    lrow = wkp.tile([1, 512], DT.float32, tag="lrow", bufs=2)
                    nc.vector.tensor_copy(lrow[:], ops[64:65, lo:hi])
                    rl = wkp.tile([1, 512], DT.float32, tag="rl", bufs=2)
                    nc.vector.reciprocal_approx_fast(out=rl[:], in_=lrow[:])
                    rlb = wkp.tile([64, 512], DT.float32, tag="rlb", bufs=2)
                    nc.gpsimd.partition_broadcast(rlb[:], rl[:])
                    nc.vector.tensor_mul(
                        oT_sb[po:po + 64, p, lo:hi], ops[0:64, lo:hi], rlb[:])

            ycopy_alt = [0]

            def emit_ywave_chunk(fcs, ec8, tag, w):
                pool = psS if tag == "scores" else psP
                yt = pool.tile([128, T], DT.float32, tag=tag,
                               name=f"yt{w}_{ec8}")
                n = len(fcs)
                for th in range(2):
                    for i, fc in enumerate(fcs):
                        nc.tensor.matmul(
                            yt[:, th * 512:(th + 1) * 512],
                            wo_sb[:, fc, ec8 * 128:(ec8 + 1) * 128],
                            oT_sb[:, fc, th * 512:(th + 1) * 512],
                            start=(i == 0), stop=(i == n - 1),
                        )
                ys = wkp.tile([128, T], MMDT, tag="ys")
                if ycopy_alt[0] % 2 == 1:
                    nc.scalar.copy(ys[:], yt[:])
                else:
                    nc.vector.tensor_copy(ys[:], yt[:])
                ycopy_alt[0] += 1
                nc.sync.dma_start(
                    youts[w][ec8 * 128:(ec8 + 1) * 128, :], ys[:])

            # ---------------- pair streams ----
            # pair 0 overlaps the remaining projections (fc1-3 + v-proj)
            emit_proj("q", 0)
            emit_proj("k", 0)
            fillers = [("p", "q", 1), ("p", "k", 1), ("p", "q", 2),
                       ("p", "k", 2), ("p", "q", 3), ("p", "k", 3)] + \
                      [("v", s) for s in range(8)]
            fi = 0
            for sc in range(8):
                emit_qk(0, sc)
                take = 2 if sc < 6 else 1
                for _ in range(take):
                    if fi < len(fillers):
                        fl = fillers[fi]; fi += 1
                        if fl[0] == "p":
                            emit_proj(fl[1], fl[2])
                        else:
                            emit_vproj(fl[1])
            while fi < len(fillers):
                fl = fillers[fi]; fi += 1
                if fl[0] == "p":
                    emit_proj(fl[1], fl[2])
                else:
                    emit_vproj(fl[1])

            # pairs 1..3: QK(p) leads, PV(p-1) trails one sc (QK priority
            # in the PE queue so throttle windows stall PV, not the exps)
            for p in range(1, 4):
                for sc in range(8):
                    emit_qk(p, sc)
                    if sc >= 1:
                        emit_pv(p - 1, sc - 1)
                    if sc == 5:
                        emit_norm(p - 1, 0)
                emit_pv(p - 1, 7)
                emit_norm(p - 1, 1)
                if p == 2:
                    for ec8 in range(8):
                        emit_ywave_chunk([0, 1], ec8, "pv", 0)

            # tail: PV(3) immediately (covers norm(2) latency), then y waves
            for i in range(8):
                emit_pv(3, i)
                if i == 4:
                    emit_norm(3, 0)
            for ec8 in range(8):
                emit_ywave_chunk([2], ec8, "scores", 1)
            emit_norm(3, 1)
            for ec8 in range(8):
                emit_ywave_chunk([3], ec8, "scores", 2)

    nc.compile()
    return nc


_NC_CACHE = []


def kernel(query, key_, value, edge_bias, attn_mask, key_padding_mask,
           Wq, bq, Wk, bk, Wv, bv, Wo, bo):
    if not _NC_CACHE:
        _NC_CACHE.append(_build_program())
    nc = _NC_CACHE[0]

    scale = np.float32(D ** -0.5)
    q32 = np.asarray(query, np.float32)
    k32 = np.asarray(key_, np.float32)
    v32 = np.asarray(value, np.float32)
    Wq32 = np.asarray(Wq, np.float32)
    Wk32 = np.asarray(Wk, np.float32)
    Wv32 = np.asarray(Wv, np.float32)
    Wo32 = np.asarray(Wo, np.float32)
    bq32 = np.asarray(bq, np.float32)
    bk32 = np.asarray(bk, np.float32)
    bv32 = np.asarray(bv, np.float32)
    bo32 = np.asarray(bo, np.float32)

    WqT = (Wq32.T * scale)
    WkT = Wk32.T
    WvT = Wv32.T
    WoT = Wo32.T

    kpm_add = np.where(np.asarray(key_padding_mask), np.float32(-1e30),
                       np.float32(0.0))          # [B, S]
    mask32 = np.asarray(attn_mask, np.float32)   # [T, S]

    def pack_x(xT):  # [E, T] -> [128, 8, T]
        return np.ascontiguousarray(
            xT.reshape(8, 128, T).transpose(1, 0, 2)).astype(NP_MMDT)

    def pack_w(wT, cols):  # [E, F-slice] -> [128, 4, 8, 128]
        w = wT[:, cols]
        w = w.reshape(8, 128, 4, 128)
        return np.ascontiguousarray(w.transpose(1, 2, 0, 3)).astype(NP_MMDT)

    in_maps = []
    for c in range(N_CORES):
        b, g = divmod(c, 2)
        cols = slice(g * F, (g + 1) * F)
        bias = (mask32.T + np.asarray(edge_bias[b], np.float32).T
                + kpm_add[b][:, None])
        wv_l = WvT[:, cols]
        in_maps.append({
            "xq": pack_x(q32[b].T),
            "xk": pack_x(k32[b].T),
            "xv": pack_x(v32[b].T),
            "wq": pack_w(WqT, cols),
            "wk": pack_w(WkT, cols),
            "wv": np.ascontiguousarray(
                wv_l.reshape(8, 128, F).transpose(1, 0, 2)).astype(NP_MMDT),
            "wo": np.ascontiguousarray(
                WoT[cols, :].reshape(4, 128, E).transpose(1, 0, 2)
            ).astype(NP_MMDT),
            "expb": np.ascontiguousarray(
                np.exp(bias).reshape(8, 128, T).transpose(1, 0, 2)
            ).astype(NP_MMDT),
            "bqd": np.ascontiguousarray(
                (scale * bq32)[cols].reshape(4, 128).T),
            "bkd": np.ascontiguousarray(bk32[cols].reshape(4, 128).T),
        })

    res = run_bass_kernel_spmd(nc, in_maps, list(range(N_CORES)))

    ycst = (bv32 @ WoT + bo32).astype(np.float32)
    out = np.empty((B, T, E), np.float32)
    for b in range(B):
        acc = np.zeros((E, T), np.float32)
        for c in (2 * b, 2 * b + 1):
            r = res.results[c]
            for w in range(3):
                acc += np.asarray(r[f"y{w}"], np.float32)
        out[b] = acc.T + ycst[None, :]
    return out


# revision 16
# speedup vs baseline: 1.1526x; 1.0345x over previous
"""Graphormer multi-head attention on 8 Trainium2 cores.

Sharding: 2 cores per batch element (B=4), each core owning 8 of 16 heads
(tensor-parallel within the batch).  Design notes (v6):

  - The exp chain is the critical path: 64 [128,1024] Exp activations on
    the Scalar engine (~1.1us each).  Everything else is arranged to hide
    under it: QK pairs feed scores through a 2-buffer PSUM rotation,
    elementwise work is balanced across DVE and Scalar, PV/projection/
    output matmuls fill the PE between QK bursts.
  - PSUM choreography (8 banks): psS "scores" = 2x[128,1024] fp32 used
    ONLY by QK scores (+ tail y-waves), so pair-0 attention starts the
    moment q/k fc0 projections finish and overlaps the DMA-paced rest of
    the projection phase.  psP "pv" = 2x[128,1024] carries proj
    accumulators -> v-proj -> PV accumulators -> mid y-waves in FIFO
    order that matches natural data readiness.
  - bq/bk ride the projection PSUM->SBUF transfer (per-partition scalar
    add); bv folds into a host-side constant (bv @ Wo.T) added at the
    end; exp(attn_mask + edge_bias) multiplies as bf16 (DVE 2x mode),
    merged over 2 score chunks per instruction.
  - Host pre-packs every input into its exact SBUF layout; one
    need-ordered DMA queue (extra queues just dilute the shared 16 DMA
    engines).  Output projection ships as 3 bf16 partials summed on host.
"""
import sys

sys.path.insert(0, '/opt/trn_rl_repo')

import ml_dtypes
import numpy as np

import concourse.bass as bass
import concourse.mybir as mybir
import concourse.tile as tile
from concourse import bacc
from concourse.bass_utils import run_bass_kernel_spmd

DT = mybir.dt
ALU = mybir.AluOpType

B, T, S, E, H = 4, 1024, 1024, 1024, 16
D = E // H          # 64
HL = 8              # heads per core
F = HL * D          # 512 local features
N_CORES = 8

MMDT = DT.bfloat16
NP_MMDT = ml_dtypes.bfloat16


def _build_program():
    nc = bacc.Bacc()

    xq = nc.dram_tensor("xq", [128, 8, T], MMDT, kind="ExternalInput")
    xk = nc.dram_tensor("xk", [128, 8, S], MMDT, kind="ExternalInput")
    xv = nc.dram_tensor("xv", [128, 8, S], MMDT, kind="ExternalInput")
    wq = nc.dram_tensor("wq", [128, 4, 8, 128], MMDT, kind="ExternalInput")
    wk = nc.dram_tensor("wk", [128, 4, 8, 128], MMDT, kind="ExternalInput")
    wv = nc.dram_tensor("wv", [128, 8, F], MMDT, kind="ExternalInput")
    wo = nc.dram_tensor("wo", [128, 4, E], MMDT, kind="ExternalInput")
    expb = nc.dram_tensor("expb", [128, 8, T], MMDT, kind="ExternalInput")
    bqd = nc.dram_tensor("bqd", [128, 4], DT.float32, kind="ExternalInput")
    bkd = nc.dram_tensor("bkd", [128, 4], DT.float32, kind="ExternalInput")
    youts = [nc.dram_tensor(f"y{w}", [E, T], MMDT, kind="ExternalOutput")
             for w in range(3)]

    with tile.TileContext(nc) as tc:
        with tc.tile_pool(name="persist", bufs=1) as pp, \
             tc.tile_pool(name="work", bufs=3) as wkp, \
             tc.tile_pool(name="big16", bufs=5) as big, \
             tc.tile_pool(name="psS", bufs=2, space="PSUM") as psS, \
             tc.tile_pool(name="psP", bufs=2, space="PSUM") as psP:

            # ---------------- DMA issues: ONE queue (sync), need-ordered ----
            wq_sb = pp.tile([128, 4, 8, 128], MMDT, tag="wq")
            xq_sb = big.tile([128, 8, T], MMDT, tag="b16")
            wk_sb = pp.tile([128, 4, 8, 128], MMDT, tag="wk")
            xk_sb = big.tile([128, 8, S], MMDT, tag="b16")
            xv_sb = big.tile([128, 8, S], MMDT, tag="b16")
            wv_sb = pp.tile([128, 8, F], MMDT, tag="wv")
            wo_sb = pp.tile([128, 4, E], MMDT, tag="wo")
            expb_sb = pp.tile([128, 8, T], MMDT, tag="expb")
            bq_sb = pp.tile([128, 4], DT.float32, tag="bq")
            bk_sb = pp.tile([128, 4], DT.float32, tag="bk")

            nc.sync.dma_start(bq_sb[:], bqd[:])
            nc.sync.dma_start(bk_sb[:], bkd[:])
            for half in range(2):
                nc.sync.dma_start(wq_sb[:, 0, 4 * half:4 * half + 4],
                                  wq[:, 0, 4 * half:4 * half + 4])
            for half in range(2):
                nc.sync.dma_start(wk_sb[:, 0, 4 * half:4 * half + 4],
                                  wk[:, 0, 4 * half:4 * half + 4])
            for ec in range(8):
                nc.sync.dma_start(xq_sb[:, ec], xq[:, ec])
                nc.sync.dma_start(xk_sb[:, ec], xk[:, ec])
            for sc in range(2):
                nc.sync.dma_start(expb_sb[:, sc], expb[:, sc])
            for fc in range(1, 4):
                nc.sync.dma_start(wq_sb[:, fc], wq[:, fc])
                nc.sync.dma_start(wk_sb[:, fc], wk[:, fc])
            for ec in range(8):
                nc.sync.dma_start(xv_sb[:, ec], xv[:, ec])
            for sc in range(2, 8):
                nc.sync.dma_start(expb_sb[:, sc], expb[:, sc])
            for ec in range(8):
                nc.sync.dma_start(wv_sb[:, ec], wv[:, ec])
            for fc in range(4):
                nc.sync.dma_start(wo_sb[:, fc], wo[:, fc])

            # ---------------- persistent activation tiles ----
            qT_sb = pp.tile([128, 4, T], MMDT, tag="qT")
            kT_sb = pp.tile([128, 4, S], MMDT, tag="kT")
            v_sb = pp.tile([128, 8, HL, 65], MMDT, tag="v")
            nc.vector.memset(v_sb[:, :, :, 64:65], 1.0)
            oT_sb = pp.tile([128, 4, T], MMDT, tag="oT")

            state = {}

            # ---------------- emission helpers ----
            def emit_proj(which, fc):
                x_sb, w_sb, b_sb, dst = {
                    "q": (xq_sb, wq_sb, bq_sb, qT_sb),
                    "k": (xk_sb, wk_sb, bk_sb, kT_sb),
                }[which]
                acc = psP.tile([128, T], DT.float32, tag="pv",
                               name=f"prj{which}{fc}")
                for th in range(2):
                    for ec in range(8):
                        nc.tensor.matmul(
                            acc[:, th * 512:(th + 1) * 512],
                            w_sb[:, fc, ec, :],
                            x_sb[:, ec, th * 512:(th + 1) * 512],
                            start=(ec == 0), stop=(ec == 7),
                        )
                nc.vector.tensor_scalar_add(
                    dst[:, fc, :], acc[:], b_sb[:, fc:fc + 1])

            def emit_vproj(sc):
                acc = psP.tile([128, F], DT.float32, tag="pv", name=f"vp{sc}")
                for ec in range(8):
                    nc.tensor.matmul(
                        acc[:],
                        xv_sb[:, ec, sc * 128:(sc + 1) * 128],
                        wv_sb[:, ec, :],
                        start=(ec == 0), stop=(ec == 7),
                    )
                nc.vector.tensor_copy(
                    v_sb[:, sc, :, 0:64],
                    acc[:].rearrange("p (h d) -> p h d", d=64),
                )

            def emit_qk(p, sc):
                if sc == 0:
                    state[(p, "pT", 0)] = big.tile([128, 8, T], MMDT,
                                                   tag="b16", name=f"pTe{p}")
                    state[(p, "pT", 1)] = big.tile([128, 8, T], MMDT,
                                                   tag="b16", name=f"pTo{p}")
                sps = [psS.tile([128, T], DT.float32, tag="scores",
                                name=f"sps{g}") for g in range(2)]
                for th in range(2):
                    for g in range(2):  # even / odd head of the pair
                        po = 64 * g
                        nc.tensor.matmul(
                            sps[g][:, th * 512:(th + 1) * 512],
                            kT_sb[po:po + 64, p, sc * 128:(sc + 1) * 128],
                            qT_sb[po:po + 64, p, th * 512:(th + 1) * 512],
                            start=True, stop=True,
                        )
                for g in range(2):
                    if sc % 2 == 0:
                        state[("et", g)] = wkp.tile(
                            [128, 2, T], MMDT, tag="et", bufs=4,
                            name=f"et{g}")
                    et = state[("et", g)]
                    nc.scalar.activation(et[:, sc % 2, :], sps[g][:],
                                         mybir.ActivationFunctionType.Exp)
                    if sc % 2 == 1:
                        nc.vector.tensor_mul(
                            state[(p, "pT", g)][:, sc - 1:sc + 1, :],
                            et[:], expb_sb[:, sc - 1:sc + 1, :],
                        )

            def emit_pv(p, i, pool=None, tag="pv"):
                if i == 0:
                    pool = pool or psP
                    state[(p, "ops", 0)] = pool.tile([65, T], DT.float32,
                                                     tag=tag, name=f"opse{p}")
                    state[(p, "ops", 1)] = pool.tile([65, T], DT.float32,
                                                     tag=tag, name=f"opso{p}")
                for j in (2 * i, 2 * i + 1):
                    th, sc = divmod(j, 8)
                    for g in range(2):
                        nc.tensor.matmul(
                            state[(p, "ops", g)][:, th * 512:(th + 1) * 512],
                            v_sb[:, sc, 2 * p + g, :],
                            state[(p, "pT", g)][:, sc, th * 512:(th + 1) * 512],
                            start=(sc == 0), stop=(sc == 7),
                        )

            def emit_norm(p, th):
                lo, hi = th * 512, (th + 1) * 512
                for g in range(2):
                    ops = state[(p, "ops", g)]
                    po = 64 * g
                    lrow = wkp.tile([1, 512], DT.float32, tag="lrow", bufs=2)
                    nc.vector.tensor_copy(lrow[:], ops[64:65, lo:hi])
                    rl = wkp.tile([1, 512], DT.float32, tag="rl", bufs=2)
                    nc.vector.reciprocal_approx_fast(out=rl[:], in_=lrow[:])
                    rlb = wkp.tile([64, 512], DT.float32, tag="rlb", bufs=2)
                    nc.gpsimd.partition_broadcast(rlb[:], rl[:])
                    nc.vector.tensor_mul(
                        oT_sb[po:po + 64, p, lo:hi], ops[0:64, lo:hi], rlb[:])

            ycopy_alt = [0]

            def emit_ywave_chunk(fcs, ec8, tag, w):
                pool = psS if tag == "scores" else psP
                yt = pool.tile([128, T], DT.float32, tag=tag,
                               name=f"yt{w}_{ec8}")
                n = len(fcs)
                for th in range(2):
                    for i, fc in enumerate(fcs):
                        nc.tensor.matmul(
                            yt[:, th * 512:(th + 1) * 512],
                            wo_sb[:, fc, ec8 * 128:(ec8 + 1) * 128],
                            oT_sb[:, fc, th * 512:(th + 1) * 512],
                            start=(i == 0), stop=(i == n - 1),
                        )
                ys = wkp.tile([128, T], MMDT, tag="ys")
                if ycopy_alt[0] % 2 == 1:
                    nc.scalar.copy(ys[:], yt[:])
                else:
                    nc.vector.tensor_copy(ys[:], yt[:])
                ycopy_alt[0] += 1
                nc.sync.dma_start(
                    youts[w][ec8 * 128:(ec8 + 1) * 128, :], ys[:])

            # ---------------- pair streams ----
            # pair 0 overlaps the remaining projections (fc1-3 + v-proj)
            emit_proj("q", 0)
            emit_proj("k", 0)
            fillers = [("p", "q", 1), ("p", "k", 1), ("p", "q", 2),
                       ("p", "k", 2), ("p", "q", 3), ("p", "k", 3)] + \
                      [("v", s) for s in range(8)]
            fi = 0
            for sc in range(8):
                emit_qk(0, sc)
                take = 2 if sc < 6 else 1
                for _ in range(take):
                    if fi < len(fillers):
                        fl = fillers[fi]; fi += 1
                        if fl[0] == "p":
                            emit_proj(fl[1], fl[2])
                        else:
                            emit_vproj(fl[1])
            while fi < len(fillers):
                fl = fillers[fi]; fi += 1
                if fl[0] == "p":
                    emit_proj(fl[1], fl[2])
                else:
                    emit_vproj(fl[1])

            # pairs 1..3: QK(p) leads, PV(p-1) trails one sc (QK priority
            # in the PE queue so throttle windows stall PV, not the exps)
            for p in range(1, 4):
                for sc in range(8):
                    emit_qk(p, sc)
                    if sc >= 1:
                        emit_pv(p - 1, sc - 1)
                    if sc == 5:
                        emit_norm(p - 1, 0)
                emit_pv(p - 1, 7)
                emit_norm(p - 1, 1)
                if p == 2:
                    for ec8 in range(8):
                        emit_ywave_chunk([0, 1], ec8, "pv", 0)

            # tail: exps are done, so the scores tag is free -> PV(3) runs
            # there immediately; y2 takes the pv tag (its slots free after
            # norm(2)); y3 follows PV(3) in the scores FIFO after norm(3)
            for i in range(8):
                emit_pv(3, i, pool=psS, tag="scores")
                if i == 4:
                    emit_norm(3, 0)
            for ec8 in range(8):
                emit_ywave_chunk([2], ec8, "pv", 1)
            emit_norm(3, 1)
            for ec8 in range(8):
                emit_ywave_chunk([3], ec8, "scores", 2)

    nc.compile()
    return nc


_NC_CACHE = []


def kernel(query, key_, value, edge_bias, attn_mask, key_padding_mask,
           Wq, bq, Wk, bk, Wv, bv, Wo, bo):
    if not _NC_CACHE:
        _NC_CACHE.append(_build_program())
    nc = _NC_CACHE[0]

    scale = np.float32(D ** -0.5)
    q32 = np.asarray(query, np.float32)
    k32 = np.asarray(key_, np.float32)
    v32 = np.asarray(value, np.float32)
    Wq32 = np.asarray(Wq, np.float32)
    Wk32 = np.asarray(Wk, np.float32)
    Wv32 = np.asarray(Wv, np.float32)
    Wo32 = np.asarray(Wo, np.float32)
    bq32 = np.asarray(bq, np.float32)
    bk32 = np.asarray(bk, np.float32)
    bv32 = np.asarray(bv, np.float32)
    bo32 = np.asarray(bo, np.float32)

    WqT = (Wq32.T * scale)
    WkT = Wk32.T
    WvT = Wv32.T
    WoT = Wo32.T

    kpm_add = np.where(np.asarray(key_padding_mask), np.float32(-1e30),
                       np.float32(0.0))          # [B, S]
    mask32 = np.asarray(attn_mask, np.float32)   # [T, S]

    def pack_x(xT):  # [E, T] -> [128, 8, T]
        return np.ascontiguousarray(
            xT.reshape(8, 128, T).transpose(1, 0, 2)).astype(NP_MMDT)

    def pack_w(wT, cols):  # [E, F-slice] -> [128, 4, 8, 128]
        w = wT[:, cols]
        w = w.reshape(8, 128, 4, 128)
        return np.ascontiguousarray(w.transpose(1, 2, 0, 3)).astype(NP_MMDT)

    in_maps = []
    for c in range(N_CORES):
        b, g = divmod(c, 2)
        cols = slice(g * F, (g + 1) * F)
        bias = (mask32.T + np.asarray(edge_bias[b], np.float32).T
                + kpm_add[b][:, None])
        wv_l = WvT[:, cols]
        in_maps.append({
            "xq": pack_x(q32[b].T),
            "xk": pack_x(k32[b].T),
            "xv": pack_x(v32[b].T),
            "wq": pack_w(WqT, cols),
            "wk": pack_w(WkT, cols),
            "wv": np.ascontiguousarray(
                wv_l.reshape(8, 128, F).transpose(1, 0, 2)).astype(NP_MMDT),
            "wo": np.ascontiguousarray(
                WoT[cols, :].reshape(4, 128, E).transpose(1, 0, 2)
            ).astype(NP_MMDT),
            "expb": np.ascontiguousarray(
                np.exp(bias).reshape(8, 128, T).transpose(1, 0, 2)
            ).astype(NP_MMDT),
            "bqd": np.ascontiguousarray(
                (scale * bq32)[cols].reshape(4, 128).T),
            "bkd": np.ascontiguousarray(bk32[cols].reshape(4, 128).T),
        })

    res = run_bass_kernel_spmd(nc, in_maps, list(range(N_CORES)))

    ycst = (bv32 @ WoT + bo32).astype(np.float32)
    out = np.empty((B, T, E), np.float32)
    for b in range(B):
        acc = np.zeros((E, T), np.float32)
        for c in (2 * b, 2 * b + 1):
            r = res.results[c]
            for w in range(3):
                acc += np.asarray(r[f"y{w}"], np.float32)
        out[b] = acc.T + ycst[None, :]
    return out


# revision 17
# speedup vs baseline: 1.1626x; 1.0087x over previous
"""Graphormer multi-head attention on 8 Trainium2 cores.

Sharding: 2 cores per batch element (B=4), each core owning 8 of 16 heads
(tensor-parallel within the batch).  Design notes (v6):

  - The exp chain is the critical path: 64 [128,1024] Exp activations on
    the Scalar engine (~1.1us each).  Everything else is arranged to hide
    under it: QK pairs feed scores through a 2-buffer PSUM rotation,
    elementwise work is balanced across DVE and Scalar, PV/projection/
    output matmuls fill the PE between QK bursts.
  - PSUM choreography (8 banks): psS "scores" = 2x[128,1024] fp32 used
    ONLY by QK scores (+ tail y-waves), so pair-0 attention starts the
    moment q/k fc0 projections finish and overlaps the DMA-paced rest of
    the projection phase.  psP "pv" = 2x[128,1024] carries proj
    accumulators -> v-proj -> PV accumulators -> mid y-waves in FIFO
    order that matches natural data readiness.
  - bq/bk ride the projection PSUM->SBUF transfer (per-partition scalar
    add); bv folds into a host-side constant (bv @ Wo.T) added at the
    end; exp(attn_mask + edge_bias) multiplies as bf16 (DVE 2x mode),
    merged over 2 score chunks per instruction.
  - Host pre-packs every input into its exact SBUF layout; one
    need-ordered DMA queue (extra queues just dilute the shared 16 DMA
    engines).  Output projection ships as 3 bf16 partials summed on host.
"""
import sys

sys.path.insert(0, '/opt/trn_rl_repo')

import ml_dtypes
import numpy as np

import concourse.bass as bass
import concourse.mybir as mybir
import concourse.tile as tile
from concourse import bacc
from concourse.bass_utils import run_bass_kernel_spmd

DT = mybir.dt
ALU = mybir.AluOpType

B, T, S, E, H = 4, 1024, 1024, 1024, 16
D = E // H          # 64
HL = 8              # heads per core
F = HL * D          # 512 local features
N_CORES = 8

MMDT = DT.bfloat16
NP_MMDT = ml_dtypes.bfloat16


def _build_program():
    nc = bacc.Bacc()

    xq = nc.dram_tensor("xq", [128, 8, T], MMDT, kind="ExternalInput")
    xk = nc.dram_tensor("xk", [128, 8, S], MMDT, kind="ExternalInput")
    xv = nc.dram_tensor("xv", [128, 8, S], MMDT, kind="ExternalInput")
    wq = nc.dram_tensor("wq", [128, 4, 8, 128], MMDT, kind="ExternalInput")
    wk = nc.dram_tensor("wk", [128, 4, 8, 128], MMDT, kind="ExternalInput")
    wv = nc.dram_tensor("wv", [128, 8, F], MMDT, kind="ExternalInput")
    wo = nc.dram_tensor("wo", [128, 4, E], MMDT, kind="ExternalInput")
    expb = nc.dram_tensor("expb", [128, 8, T], MMDT, kind="ExternalInput")
    bqd = nc.dram_tensor("bqd", [128, 4], DT.float32, kind="ExternalInput")
    bkd = nc.dram_tensor("bkd", [128, 4], DT.float32, kind="ExternalInput")
    youts = [nc.dram_tensor(f"y{w}", [E, T], MMDT, kind="ExternalOutput")
             for w in range(3)]

    with tile.TileContext(nc) as tc:
        with tc.tile_pool(name="persist", bufs=1) as pp, \
             tc.tile_pool(name="work", bufs=3) as wkp, \
             tc.tile_pool(name="big16", bufs=5) as big, \
             tc.tile_pool(name="psS", bufs=2, space="PSUM") as psS, \
             tc.tile_pool(name="psP", bufs=2, space="PSUM") as psP:

            # ---------------- DMA issues: ONE queue (sync), need-ordered ----
            wq_sb = pp.tile([128, 4, 8, 128], MMDT, tag="wq")
            xq_sb = big.tile([128, 8, T], MMDT, tag="b16")
            wk_sb = pp.tile([128, 4, 8, 128], MMDT, tag="wk")
            xk_sb = big.tile([128, 8, S], MMDT, tag="b16")
            xv_sb = big.tile([128, 8, S], MMDT, tag="b16")
            wv_sb = pp.tile([128, 8, F], MMDT, tag="wv")
            wo_sb = pp.tile([128, 4, E], MMDT, tag="wo")
            expb_sb = pp.tile([128, 8, T], MMDT, tag="expb")
            bq_sb = pp.tile([128, 4], DT.float32, tag="bq")
            bk_sb = pp.tile([128, 4], DT.float32, tag="bk")

            nc.sync.dma_start(bq_sb[:], bqd[:])
            nc.sync.dma_start(bk_sb[:], bkd[:])
            for half in range(2):
                nc.sync.dma_start(wq_sb[:, 0, 4 * half:4 * half + 4],
                                  wq[:, 0, 4 * half:4 * half + 4])
            for half in range(2):
                nc.sync.dma_start(wk_sb[:, 0, 4 * half:4 * half + 4],
                                  wk[:, 0, 4 * half:4 * half + 4])
            for e2 in range(4):
                nc.sync.dma_start(xq_sb[:, 2 * e2:2 * e2 + 2],
                                  xq[:, 2 * e2:2 * e2 + 2])
                nc.sync.dma_start(xk_sb[:, 2 * e2:2 * e2 + 2],
                                  xk[:, 2 * e2:2 * e2 + 2])
            nc.sync.dma_start(expb_sb[:, 0:2], expb[:, 0:2])
            for fc in range(1, 4):
                nc.sync.dma_start(wq_sb[:, fc], wq[:, fc])
                nc.sync.dma_start(wk_sb[:, fc], wk[:, fc])
            for e2 in range(4):
                nc.sync.dma_start(xv_sb[:, 2 * e2:2 * e2 + 2],
                                  xv[:, 2 * e2:2 * e2 + 2])
            for s2 in range(1, 4):
                nc.sync.dma_start(expb_sb[:, 2 * s2:2 * s2 + 2],
                                  expb[:, 2 * s2:2 * s2 + 2])
            for ec in range(8):
                nc.sync.dma_start(wv_sb[:, ec], wv[:, ec])
            for fc in range(4):
                nc.sync.dma_start(wo_sb[:, fc], wo[:, fc])

            # ---------------- persistent activation tiles ----
            qT_sb = pp.tile([128, 4, T], MMDT, tag="qT")
            kT_sb = pp.tile([128, 4, S], MMDT, tag="kT")
            v_sb = pp.tile([128, 8, HL, 65], MMDT, tag="v")
            nc.vector.memset(v_sb[:, :, :, 64:65], 1.0)
            oT_sb = pp.tile([128, 4, T], MMDT, tag="oT")

            state = {}

            # ---------------- emission helpers ----
            def emit_proj(which, fc):
                x_sb, w_sb, b_sb, dst = {
                    "q": (xq_sb, wq_sb, bq_sb, qT_sb),
                    "k": (xk_sb, wk_sb, bk_sb, kT_sb),
                }[which]
                acc = psP.tile([128, T], DT.float32, tag="pv",
                               name=f"prj{which}{fc}")
                for th in range(2):
                    for ec in range(8):
                        nc.tensor.matmul(
                            acc[:, th * 512:(th + 1) * 512],
                            w_sb[:, fc, ec, :],
                            x_sb[:, ec, th * 512:(th + 1) * 512],
                            start=(ec == 0), stop=(ec == 7),
                        )
                nc.vector.tensor_scalar_add(
                    dst[:, fc, :], acc[:], b_sb[:, fc:fc + 1])

            def emit_vproj(sc):
                acc = psP.tile([128, F], DT.float32, tag="pv", name=f"vp{sc}")
                for ec in range(8):
                    nc.tensor.matmul(
                        acc[:],
                        xv_sb[:, ec, sc * 128:(sc + 1) * 128],
                        wv_sb[:, ec, :],
                        start=(ec == 0), stop=(ec == 7),
                    )
                nc.vector.tensor_copy(
                    v_sb[:, sc, :, 0:64],
                    acc[:].rearrange("p (h d) -> p h d", d=64),
                )

            def emit_qk(p, sc):
                if sc == 0:
                    state[(p, "pT", 0)] = big.tile([128, 8, T], MMDT,
                                                   tag="b16", name=f"pTe{p}")
                    state[(p, "pT", 1)] = big.tile([128, 8, T], MMDT,
                                                   tag="b16", name=f"pTo{p}")
                sps = [psS.tile([128, T], DT.float32, tag="scores",
                                name=f"sps{g}") for g in range(2)]
                for th in range(2):
                    for g in range(2):  # even / odd head of the pair
                        po = 64 * g
                        nc.tensor.matmul(
                            sps[g][:, th * 512:(th + 1) * 512],
                            kT_sb[po:po + 64, p, sc * 128:(sc + 1) * 128],
                            qT_sb[po:po + 64, p, th * 512:(th + 1) * 512],
                            start=True, stop=True,
                        )
                for g in range(2):
                    if sc % 2 == 0:
                        state[("et", g)] = wkp.tile(
                            [128, 2, T], MMDT, tag="et", bufs=4,
                            name=f"et{g}")
                    et = state[("et", g)]
                    nc.scalar.activation(et[:, sc % 2, :], sps[g][:],
                                         mybir.ActivationFunctionType.Exp)
                    if sc % 2 == 1:
                        nc.vector.tensor_mul(
                            state[(p, "pT", g)][:, sc - 1:sc + 1, :],
                            et[:], expb_sb[:, sc - 1:sc + 1, :],
                        )

            def emit_pv(p, i, pool=None, tag="pv"):
                if i == 0:
                    pool = pool or psP
                    state[(p, "ops", 0)] = pool.tile([65, T], DT.float32,
                                                     tag=tag, name=f"opse{p}")
                    state[(p, "ops", 1)] = pool.tile([65, T], DT.float32,
                                                     tag=tag, name=f"opso{p}")
                for j in (2 * i, 2 * i + 1):
                    th, sc = divmod(j, 8)
                    for g in range(2):
                        nc.tensor.matmul(
                            state[(p, "ops", g)][:, th * 512:(th + 1) * 512],
                            v_sb[:, sc, 2 * p + g, :],
                            state[(p, "pT", g)][:, sc, th * 512:(th + 1) * 512],
                            start=(sc == 0), stop=(sc == 7),
                        )

            def emit_norm(p, th):
                lo, hi = th * 512, (th + 1) * 512
                for g in range(2):
                    ops = state[(p, "ops", g)]
                    po = 64 * g
                    lrow = wkp.tile([1, 512], DT.float32, tag="lrow", bufs=2)
                    nc.vector.tensor_copy(lrow[:], ops[64:65, lo:hi])
                    rl = wkp.tile([1, 512], DT.float32, tag="rl", bufs=2)
                    nc.vector.reciprocal_approx_fast(out=rl[:], in_=lrow[:])
                    rlb = wkp.tile([64, 512], DT.float32, tag="rlb", bufs=2)
                    nc.gpsimd.partition_broadcast(rlb[:], rl[:])
                    nc.vector.tensor_mul(
                        oT_sb[po:po + 64, p, lo:hi], ops[0:64, lo:hi], rlb[:])

            ycopy_alt = [0]

            def emit_ywave_chunk(fcs, ec8, tag, w):
                pool = psS if tag == "scores" else psP
                yt = pool.tile([128, T], DT.float32, tag=tag,
                               name=f"yt{w}_{ec8}")
                n = len(fcs)
                for th in range(2):
                    for i, fc in enumerate(fcs):
                        nc.tensor.matmul(
                            yt[:, th * 512:(th + 1) * 512],
                            wo_sb[:, fc, ec8 * 128:(ec8 + 1) * 128],
                            oT_sb[:, fc, th * 512:(th + 1) * 512],
                            start=(i == 0), stop=(i == n - 1),
                        )
                ys = wkp.tile([128, T], MMDT, tag="ys")
                if w > 0 and ycopy_alt[0] % 2 == 1:
                    nc.scalar.copy(ys[:], yt[:])  # tail only: scalar is idle
                else:
                    nc.vector.tensor_copy(ys[:], yt[:])
                ycopy_alt[0] += 1
                nc.sync.dma_start(
                    youts[w][ec8 * 128:(ec8 + 1) * 128, :], ys[:])

            # ---------------- pair streams ----
            # pair 0 overlaps the remaining projections (fc1-3 + v-proj)
            emit_proj("q", 0)
            emit_proj("k", 0)
            fillers = [("p", "q", 1), ("p", "k", 1), ("p", "q", 2),
                       ("p", "k", 2), ("p", "q", 3), ("p", "k", 3)] + \
                      [("v", s) for s in range(8)]
            fi = 0
            for sc in range(8):
                emit_qk(0, sc)
                take = 2 if sc < 6 else 1
                for _ in range(take):
                    if fi < len(fillers):
                        fl = fillers[fi]; fi += 1
                        if fl[0] == "p":
                            emit_proj(fl[1], fl[2])
                        else:
                            emit_vproj(fl[1])
            while fi < len(fillers):
                fl = fillers[fi]; fi += 1
                if fl[0] == "p":
                    emit_proj(fl[1], fl[2])
                else:
                    emit_vproj(fl[1])

            # pairs 1..3: QK(p) leads, PV(p-1) trails one sc (QK priority
            # in the PE queue so throttle windows stall PV, not the exps)
            for p in range(1, 4):
                for sc in range(8):
                    emit_qk(p, sc)
                    if sc >= 1:
                        emit_pv(p - 1, sc - 1)
                    if sc == 5:
                        emit_norm(p - 1, 0)
                emit_pv(p - 1, 7)
                emit_norm(p - 1, 1)
                if p == 2:
                    for ec8 in range(8):
                        emit_ywave_chunk([0, 1], ec8, "pv", 0)

            # tail: exps are done, so the scores tag is free -> PV(3) runs
            # there immediately; y2 takes the pv tag (its slots free after
            # norm(2)); y3 follows PV(3) in the scores FIFO after norm(3)
            for i in range(8):
                emit_pv(3, i, pool=psS, tag="scores")
                if i == 4:
                    emit_norm(3, 0)
            for ec8 in range(8):
                emit_ywave_chunk([2], ec8, "pv", 1)
            emit_norm(3, 1)
            for ec8 in range(8):
                emit_ywave_chunk([3], ec8, "scores", 2)

    nc.compile()
    return nc


_NC_CACHE = []


def kernel(query, key_, value, edge_bias, attn_mask, key_padding_mask,
           Wq, bq, Wk, bk, Wv, bv, Wo, bo):
    if not _NC_CACHE:
        _NC_CACHE.append(_build_program())
    nc = _NC_CACHE[0]

    scale = np.float32(D ** -0.5)
    q32 = np.asarray(query, np.float32)
    k32 = np.asarray(key_, np.float32)
    v32 = np.asarray(value, np.float32)
    Wq32 = np.asarray(Wq, np.float32)
    Wk32 = np.asarray(Wk, np.float32)
    Wv32 = np.asarray(Wv, np.float32)
    Wo32 = np.asarray(Wo, np.float32)
    bq32 = np.asarray(bq, np.float32)
    bk32 = np.asarray(bk, np.float32)
    bv32 = np.asarray(bv, np.float32)
    bo32 = np.asarray(bo, np.float32)

    WqT = (Wq32.T * scale)
    WkT = Wk32.T
    WvT = Wv32.T
    WoT = Wo32.T

    kpm_add = np.where(np.asarray(key_padding_mask), np.float32(-1e30),
                       np.float32(0.0))          # [B, S]
    mask32 = np.asarray(attn_mask, np.float32)   # [T, S]

    def pack_x(xT):  # [E, T] -> [128, 8, T]
        return np.ascontiguousarray(
            xT.reshape(8, 128, T).transpose(1, 0, 2)).astype(NP_MMDT)

    def pack_w(wT, cols):  # [E, F-slice] -> [128, 4, 8, 128]
        w = wT[:, cols]
        w = w.reshape(8, 128, 4, 128)
        return np.ascontiguousarray(w.transpose(1, 2, 0, 3)).astype(NP_MMDT)

    in_maps = []
    for c in range(N_CORES):
        b, g = divmod(c, 2)
        cols = slice(g * F, (g + 1) * F)
        bias = (mask32.T + np.asarray(edge_bias[b], np.float32).T
                + kpm_add[b][:, None])
        wv_l = WvT[:, cols]
        in_maps.append({
            "xq": pack_x(q32[b].T),
            "xk": pack_x(k32[b].T),
            "xv": pack_x(v32[b].T),
            "wq": pack_w(WqT, cols),
            "wk": pack_w(WkT, cols),
            "wv": np.ascontiguousarray(
                wv_l.reshape(8, 128, F).transpose(1, 0, 2)).astype(NP_MMDT),
            "wo": np.ascontiguousarray(
                WoT[cols, :].reshape(4, 128, E).transpose(1, 0, 2)
            ).astype(NP_MMDT),
            "expb": np.ascontiguousarray(
                np.exp(bias).reshape(8, 128, T).transpose(1, 0, 2)
            ).astype(NP_MMDT),
            "bqd": np.ascontiguousarray(
                (scale * bq32)[cols].reshape(4, 128).T),
            "bkd": np.ascontiguousarray(bk32[cols].reshape(4, 128).T),
        })

    res = run_bass_kernel_spmd(nc, in_maps, list(range(N_CORES)))

    ycst = (bv32 @ WoT + bo32).astype(np.float32)
    out = np.empty((B, T, E), np.float32)
    for b in range(B):
        acc = np.zeros((E, T), np.float32)
        for c in (2 * b, 2 * b + 1):
            r = res.results[c]
            for w in range(3):
                acc += np.asarray(r[f"y{w}"], np.float32)
        out[b] = acc.T + ycst[None, :]
    return out
